# revision 2
# baseline (speedup 1.0000x reference)
"""Trainium2 Bass kernel for soft tree-gated MoE routing (nn_FFF_66958540145282).

Reference computation (fp32):
  mixture[t, leaf] = prod over tree depth of sigmoid gates  (soft routing)
  h_l = relu(x @ w1s[l] + b1s[l]);  y = sum_l (h_l @ w2s[l] + b2s[l]) * mixture[:, l]

Sharding: data-parallel over the 8192-token batch across 8 NeuronCores
(1024 tokens/core); all parameter tensors replicated. No collectives.

Per-core kernel layout (tokens t are local):
  xT[f, t]   : x transposed on-chip via PE transposes (needed as the moving
               operand for MM1 and as lhsT for routing logits).
  MM1        : hT_l[h, t] = w1_l.T @ x.T -- lhsT = w1 chunk [128F, 128H],
               rhs = xT [128F, 512t].  ReLU+bias fused on ACT (bias is
               per-partition in the hT layout).
  mixture    : logits via PE, sigmoid(+/-) on ACT, tree product on DVE,
               transposed to mT[leaf, t] via PE.
  scale      : aT_l = relu(hT_l + b1) * m_bcast, where m_bcast[p, t] =
               mixture[t, l] for all p comes from a selector matmul
               (exact copy through the PE).
  MM2        : y[t, o] accumulated in PSUM over all 16 leaves;
               initialized by the bias matmul mixture @ b2s (K=16).

Matmul dtypes: float32r (fp32 stored, PE rounds to 1-8-11 ~tf32, full rate
at free dim >= 256) and/or bf16.  float32r operands must be *produced* as
float32r for the walrus verifier, so tensors are declared float32r
end-to-end and host arrays are pre-rounded to the 1-8-11 grid.
"""

import numpy as np
import ml_dtypes
from dataclasses import dataclass

B, F, H, O, L, NN, DEPTH = 8192, 1024, 256, 1024, 16, 15, 4
NP = 16               # routing matmul free dim (15 padded to even 16 for f32r)
NCORES = 8
BC = B // NCORES      # tokens per core
P = 128
FC = F // P           # 8 feature chunks
HC = H // P           # 2 hidden chunks
MOV = 512             # moving-dim (free) size for matmuls


@dataclass(frozen=True)
class Cfg:
    mm1: str = "f32r"       # dtype for x@w1 (and routing) matmuls: f32 | f32r | bf16
    mm2: str = "f32r"       # dtype for a@w2 matmuls (aT / w2 storage)
    bcast: str = "gps"      # mixture broadcast: "gps" (exact, via gpsimd),
                            # "mm2" (selector matmul in mm2 dtype), or "f32"
    xt_host: bool = True    # x transposed on host (input becomes [P, FC, BC])
    w1_bufs: int = 2        # w1 stream pool depth
    w2_bufs: int = 16       # w2 resident pool depth (16 live + extra = prefetch)
    repeat: int = 1         # how many times the full body runs (timing builds)
    use_for_i: bool = False  # hardware loop instead of python unroll


DEFAULT_CFG = Cfg()

_BUILD_CACHE: dict = {}


def _round_f32r(a: np.ndarray) -> np.ndarray:
    """Round fp32 to the float32r (1-8-11) grid, round-to-nearest-even."""
    u = np.ascontiguousarray(a, np.float32).view(np.uint32)
    lsb = (u >> np.uint32(12)) & np.uint32(1)
    r = (u + np.uint32(0x7FF) + lsb) & np.uint32(0xFFFFF000)
    return r.view(np.float32)


def _np_cast(a, kind):
    if kind == "bf16":
        return np.ascontiguousarray(np.asarray(a).astype(ml_dtypes.bfloat16))
    if kind == "f32r":
        return np.ascontiguousarray(_round_f32r(np.asarray(a)))
    return np.ascontiguousarray(np.asarray(a, np.float32))


def _np_dt(kind):
    return ml_dtypes.bfloat16 if kind == "bf16" else np.float32


def build(cfg: Cfg):
    import concourse.mybir as mybir
    import concourse.tile as tile
    from concourse import bacc

    f32 = mybir.dt.float32
    AF = mybir.ActivationFunctionType
    MUL = mybir.AluOpType.mult

    def dt_of(kind):
        if kind == "bf16":
            return mybir.dt.bfloat16
        if kind == "f32r":
            return mybir.dt.float32r
        return f32

    x_kind = cfg.mm1
    a_kind = cfg.mm2
    gps = cfg.bcast == "gps"
    # with gps broadcast, mT itself stays fp32 (gpsimd datapath) and only the
    # small bias-matmul copy mT_r is in the mm2 dtype
    m_kind = "f32" if gps else (cfg.mm2 if cfg.bcast == "mm2" else "f32")
    mr_kind = cfg.mm2 if gps else m_kind
    x_dt = dt_of(x_kind)
    a_dt = dt_of(a_kind)
    m_dt = dt_of(m_kind)
    mr_dt = dt_of(mr_kind)

    # 2 token passes when aT must be stored at 4 bytes (SBUF pressure), else 1.
    n_pass = 1 if a_kind == "bf16" else 2
    TPB = BC // n_pass      # tokens per pass
    TG = TPB // MOV         # moving-dim groups per pass
    TCC = TPB // P          # 128-token tiles per pass
    OG = O // MOV           # output column halves

    nc = bacc.Bacc("TRN2", target_bir_lowering=False, debug=False,
                   num_devices=NCORES)

    if cfg.xt_host:
        x_d = nc.dram_tensor("x", [P, FC, BC], x_dt, kind="ExternalInput")
    else:
        x_d = nc.dram_tensor("x", [BC, F], x_dt, kind="ExternalInput")
    nwt_d = nc.dram_tensor("nwt", [P, FC, NP], x_dt, kind="ExternalInput")
    nbt_d = nc.dram_tensor("nbt", [16, NP], x_dt, kind="ExternalInput")
    w1_d = nc.dram_tensor("w1", [L, F, H], x_dt, kind="ExternalInput")
    b1t_d = nc.dram_tensor("b1t", [P, L, HC], f32, kind="ExternalInput")
    w2_d = nc.dram_tensor("w2", [L, H, O], a_dt, kind="ExternalInput")
    b2_d = nc.dram_tensor("b2", [L, O], mr_dt, kind="ExternalInput")
    selx_d = nc.dram_tensor("selx", [16, L, P], x_dt, kind="ExternalInput")
    selm_d = nc.dram_tensor("selm", [16, L, P], m_dt, kind="ExternalInput")
    idx_d = nc.dram_tensor("identx", [P, P], x_dt, kind="ExternalInput")
    idf_d = nc.dram_tensor("identf", [P, P], f32, kind="ExternalInput")
    y_d = nc.dram_tensor("y", [BC, O], f32, kind="ExternalOutput")

    with tile.TileContext(nc) as tc:
        with (
            tc.tile_pool(name="const", bufs=1) as const,
            tc.tile_pool(name="big", bufs=1) as big,
            tc.tile_pool(name="w1p", bufs=cfg.w1_bufs) as w1p,
            tc.tile_pool(name="w2p", bufs=cfg.w2_bufs) as w2p,
            tc.tile_pool(name="xload", bufs=2) as xloadp,
            tc.tile_pool(name="small", bufs=2) as small,
            tc.tile_pool(name="work", bufs=3) as work,
            tc.tile_pool(name="mbcp", bufs=2) as mbcp,
            tc.tile_pool(name="ph", bufs=(5 if cfg.bcast == "gps" else 3),
                         space="PSUM") as ph,
            tc.tile_pool(name="pm", bufs=2, space="PSUM") as pm,
            tc.tile_pool(name="py", bufs=3, space="PSUM") as py,
        ):
            # ---------------- constants ----------------
            ident_x = const.tile([P, P], x_dt)
            nc.sync.dma_start(ident_x, idx_d[:])
            if x_dt == f32:
                ident_f = ident_x
            else:
                ident_f = const.tile([P, P], f32)
                nc.sync.dma_start(ident_f, idf_d[:])
            nwt = const.tile([P, FC, NP], x_dt)
            nc.sync.dma_start(nwt, nwt_d[:])
            nbt16 = const.tile([16, NP], x_dt)
            nc.sync.dma_start(nbt16, nbt_d[:])
            b1t = const.tile([P, L, HC], f32)
            nc.sync.dma_start(b1t, b1t_d[:])
            b2t = const.tile([L, O], mr_dt)
            nc.sync.dma_start(b2t, b2_d[:])
            # Selector tiles: sel[k, l, :] = 1.0 if k == l else 0 -- used as
            # lhsT so that matmul(sel[:, l, :], rhs[16, N]) == rhs[l] broadcast
            # to 128 partitions, exactly.
            sel_x = const.tile([16, L, P], x_dt)
            nc.sync.dma_start(sel_x, selx_d[:])
            if gps:
                sel_m = None  # selector matmul replaced by gpsimd broadcast
            else:
                sel_m = const.tile([16, L, P], m_dt)
                nc.sync.dma_start(sel_m, selm_d[:])

            def rep_body(_i=None):
                for ps in range(n_pass):
                    t0 = ps * TPB
                    xT = big.tile([P, FC, TPB], x_dt, tag="xT")
                    aT = big.tile([P, L, HC, TPB], a_dt, tag="aT")
                    mixs = small.tile([P, TCC, L], f32, tag="mixs")
                    gp = small.tile([P, TCC, NN], f32, tag="gp")
                    gn = small.tile([P, TCC, NN], f32, tag="gn")
                    mT = small.tile([16, TPB], m_dt, tag="mT")
                    if gps:
                        mT_r = small.tile([16, TPB], mr_dt, tag="mTr", name="mTr")
                    else:
                        mT_r = mT

                    # ---- setup: transpose x, routing, mixture ----
                    if cfg.xt_host:
                        nc.sync.dma_start(xT, x_d[:, :, t0:t0 + TPB])
                    else:
                        for tcc in range(TCC):
                            trow = slice(t0 + tcc * P, t0 + (tcc + 1) * P)
                            xl = xloadp.tile([P, F], x_dt, tag="xload")
                            nc.sync.dma_start(xl, x_d[trow, :])
                            for fc in range(FC):
                                pt = ph.tile([P, MOV], x_dt, tag="ph")
                                nc.tensor.transpose(
                                    pt[:, :P], xl[:, fc * P:(fc + 1) * P], ident_x)
                                nc.vector.tensor_copy(
                                    xT[:, fc, tcc * P:(tcc + 1) * P], pt[:, :P])

                    for tcc in range(TCC):
                        tsl = slice(tcc * P, (tcc + 1) * P)
                        lg = ph.tile([P, MOV], f32, tag="ph")
                        lgs = lg[:, :NP]
                        for fc in range(FC):
                            nc.tensor.matmul(
                                lgs, xT[:, fc, tsl], nwt[:, fc, :],
                                start=(fc == 0), stop=False)
                        nc.tensor.matmul(
                            lgs, sel_x[:, 0, :], nbt16,
                            start=False, stop=True)
                        nc.scalar.activation(gp[:, tcc, :], lg[:, :NN],
                                             AF.Sigmoid)
                        nc.scalar.activation(gn[:, tcc, :], lg[:, :NN],
                                             AF.Sigmoid, scale=-1.0)

                        # tree product: mixture[t, l]
                        t2 = small.tile([P, 2], f32, tag="t2")
                        nc.vector.tensor_copy(t2[:, 0:1], gn[:, tcc, 0:1])
                        nc.vector.tensor_copy(t2[:, 1:2], gp[:, tcc, 0:1])
                        prev = t2
                        for d in (1, 2, 3):
                            n = 1 << d
                            lo = n - 1
                            gi = small.tile([P, n, 2], f32, tag=f"gi{n}")
                            nc.vector.tensor_copy(gi[:, :, 0], gn[:, tcc, lo:lo + n])
                            nc.vector.tensor_copy(gi[:, :, 1], gp[:, tcc, lo:lo + n])
                            if d < 3:
                                cur = small.tile([P, 2 * n], f32, tag=f"t{2 * n}")
                                dst = cur.rearrange("p (i j) -> p i j", j=2)
                            else:
                                cur = None
                                dst = mixs[:, tcc, :].rearrange(
                                    "p (i j) -> p i j", j=2)
                            nc.vector.tensor_tensor(
                                dst, prev[:, :, None].to_broadcast((P, n, 2)),
                                gi, MUL)
                            prev = cur

                        pmt = ph.tile([P, MOV], f32, tag="ph")
                        nc.tensor.transpose(pmt[:L, :P], mixs[:, tcc, :], ident_f)
                        nc.vector.tensor_copy(mT[:, tsl], pmt[:L, :P])
                        if gps:
                            nc.vector.tensor_copy(mT_r[:, tsl], pmt[:L, :P])

                    # ---- phase 1: aT[l] = relu(w1.T x.T + b1) * mixture ----
                    for l in range(L):
                        w1sb = w1p.tile([P, FC, H], x_dt, tag="w1")
                        nc.sync.dma_start(
                            w1sb, w1_d[l].rearrange("(fc p) h -> p fc h", p=P))
                        for g in range(TG):
                            gs = slice(g * MOV, (g + 1) * MOV)
                            if gps:
                                mrow = mbcp.tile([1, MOV], f32, tag="mrow")
                                nc.sync.dma_start(mrow, mT[l:l + 1, gs])
                                mp = mbcp.tile([P, MOV], f32, tag="mbc")
                                nc.gpsimd.partition_broadcast(mp, mrow)
                            else:
                                mp = pm.tile([P, MOV], f32, tag="pm")
                                nc.tensor.matmul(
                                    mp, sel_m[:, l, :], mT[:, gs],
                                    start=True, stop=True)
                            for hc in range(HC):
                                hp = ph.tile([P, MOV], f32, tag="ph")
                                for fc in range(FC):
                                    nc.tensor.matmul(
                                        hp,
                                        w1sb[:, fc, hc * P:(hc + 1) * P],
                                        xT[:, fc, gs],
                                        start=(fc == 0), stop=(fc == FC - 1))
                                ar = work.tile([P, MOV], f32, tag="ar")
                                nc.scalar.activation(
                                    ar, hp, AF.Relu, bias=b1t[:, l, hc:hc + 1])
                                nc.vector.tensor_tensor(
                                    aT[:, l, hc, gs], ar, mp, MUL)

                    # ---- phase 2: y = sum_l aT_l.T @ w2_l  (+ mixture @ b2) ----
                    for og in range(OG):
                        ogs = slice(og * MOV, (og + 1) * MOV)
                        w2t = []
                        for l in range(L):
                            w = w2p.tile([P, HC, MOV], a_dt, tag="w2")
                            nc.sync.dma_start(
                                w, w2_d[l].rearrange("(hc p) o -> p hc o", p=P)[:, :, ogs])
                            w2t.append(w)
                        for tcc in range(TCC):
                            tsl = slice(tcc * P, (tcc + 1) * P)
                            trow = slice(t0 + tcc * P, t0 + (tcc + 1) * P)
                            yp = py.tile([P, MOV], f32, tag="py")
                            nc.tensor.matmul(
                                yp, mT_r[:, tsl], b2t[:, ogs],
                                start=True, stop=False)
                            for l in range(L):
                                for hc in range(HC):
                                    nc.tensor.matmul(
                                        yp,
                                        aT[:, l, hc, tsl],
                                        w2t[l][:, hc, :],
                                        start=False,
                                        stop=(l == L - 1 and hc == HC - 1))
                            ye = work.tile([P, MOV], f32, tag="ye")
                            nc.vector.tensor_copy(ye, yp)
                            nc.sync.dma_start(y_d[trow, ogs], ye)

            if cfg.use_for_i:
                with tc.For_i(0, cfg.repeat, 1) as _i:
                    rep_body(_i)
            else:
                for _ in range(cfg.repeat):
                    rep_body()

    nc.compile()
    return nc


def host_prep(inputs, cfg: Cfg):
    """Shared (replicated) parameter arrays in kernel layout."""
    node_weights = np.asarray(inputs["node_weights"], np.float32)
    node_biases = np.asarray(inputs["node_biases"], np.float32)
    w1s = np.asarray(inputs["w1s"], np.float32)
    b1s = np.asarray(inputs["b1s"], np.float32)
    w2s = np.asarray(inputs["w2s"], np.float32)
    b2s = np.asarray(inputs["b2s"], np.float32)

    m_kind = "f32" if cfg.bcast == "gps" else (
        cfg.mm2 if cfg.bcast == "mm2" else "f32")
    mr_kind = cfg.mm2 if cfg.bcast == "gps" else m_kind
    nwt = np.zeros((P, FC, NP), np.float32)
    nwt[:, :, :NN] = node_weights.T.reshape(FC, P, NN).transpose(1, 0, 2)
    nbt = np.zeros((16, NP), np.float32)
    nbt[0, :NN] = node_biases.reshape(NN)
    b1t = b1s.reshape(L, HC, P).transpose(2, 0, 1)              # [P, L, HC]
    sel = np.zeros((16, L, P), np.float32)
    sel[np.arange(L), np.arange(L), :] = 1.0
    return {
        "nwt": _np_cast(nwt, cfg.mm1),
        "nbt": _np_cast(nbt, cfg.mm1),
        "w1": _np_cast(w1s, cfg.mm1),
        "b1t": np.ascontiguousarray(b1t),
        "w2": _np_cast(w2s, cfg.mm2),
        "b2": _np_cast(b2s, mr_kind),
        "selx": _np_cast(sel, cfg.mm1),
        "selm": _np_cast(sel, m_kind),
        "identx": _np_cast(np.eye(P, dtype=np.float32), cfg.mm1),
        "identf": np.ascontiguousarray(np.eye(P, dtype=np.float32)),
    }


def get_nc(cfg: Cfg):
    if cfg not in _BUILD_CACHE:
        _BUILD_CACHE[cfg] = build(cfg)
    return _BUILD_CACHE[cfg]


def prep_x(x, cfg: Cfg):
    """Per-core x shards in kernel layout."""
    xc = _np_cast(np.asarray(x, np.float32), cfg.mm1)

    def shard(c):
        xs = xc[c * BC:(c + 1) * BC]
        if cfg.xt_host:
            xs = xs.reshape(BC, FC, P).transpose(2, 1, 0)  # [P, FC, BC]
        return np.ascontiguousarray(xs)

    return [shard(c) for c in range(NCORES)]


def run(inputs, cfg: Cfg):
    from concourse.bass_utils import run_bass_kernel_spmd

    nc = get_nc(cfg)
    params = host_prep(inputs, cfg)
    xshards = prep_x(inputs["x"], cfg)
    in_maps = [{"x": xshards[c], **params} for c in range(NCORES)]
    res = run_bass_kernel_spmd(nc, in_maps, core_ids=list(range(NCORES)))
    y = np.concatenate([res.results[c]["y"] for c in range(NCORES)], axis=0)
    return np.ascontiguousarray(y.astype(np.float32))


def kernel(**inputs) -> np.ndarray:
    return run(inputs, DEFAULT_CFG)



# revision 3
# speedup vs baseline: 1.0550x; 1.0550x over previous
"""Trainium2 Bass kernel v2 for soft tree-gated MoE routing (nn_FFF_66958540145282).

Reference computation (fp32):
  mixture[t, leaf] = prod over tree depth of sigmoid gates  (soft routing)
  h_l = relu(x @ w1s[l] + b1s[l]);  y = sum_l (h_l @ w2s[l] + b2s[l]) * mixture[:, l]

Sharding: data-parallel over the 8192-token batch across 8 NeuronCores
(1024 tokens/core); all parameter tensors replicated. No collectives.

v2 changes vs v1:
  * mm2 (aT/w2) in bf16 -> single token pass, w2 fully resident, half DMA.
  * routing in log space: mT[l, t] = exp(-(A @ softplus(+/-(x@w_n + b_n)))[l, t])
    computed directly in [node, token] layout (logits matmul with nwt as
    stationary operand) -- no tree-product DVE chain, no PE transposes.
  * mixture broadcast via gpsimd.partition_broadcast reading mT[l] directly
    (no SBUF->SBUF staging DMA), or a selector matmul ("mm2" mode).
  * relu+bias applied in-place in PSUM by ACT; DVE multiplies PSUM*mp -> aT.
  * w1/w2/x pre-arranged on host so every DMA is partition-contiguous.
"""

import numpy as np
import ml_dtypes
from dataclasses import dataclass

B, F, H, O, L, NN, DEPTH = 8192, 1024, 256, 1024, 16, 15, 4
NCORES = 8
BC = B // NCORES      # tokens per core
P = 128
FC = F // P           # 8 feature chunks
HC = H // P           # 2 hidden chunks
MOV = 512             # moving-dim (free) size for matmuls
OG = O // MOV         # output column halves


@dataclass(frozen=True)
class Cfg:
    mm1: str = "bf16"       # dtype for x/w1/routing matmuls: f32r | bf16
    mm2: str = "bf16"       # dtype for aT/w2 matmuls: f32r | bf16
    bcast: str = "gps"      # gps (staged DMA + gpsimd) | mm2 (selector matmul)
    relu_inplace: bool = True
    w1_bufs: int = 4
    w2_bufs: int = 16       # resident bf16 w2 (16 live)
    mbc_bufs: int = 2
    work_bufs: int = 2
    y_bf16: bool = False    # store y output in bf16 (halves y DMA traffic)
    repeat: int = 1
    use_for_i: bool = False
    diag_nodma: bool = False   # timing diagnostic: skip xT/w1/w2 streaming


DEFAULT_CFG = Cfg()

_BUILD_CACHE: dict = {}


def _round_f32r(a: np.ndarray) -> np.ndarray:
    """Round fp32 to the float32r (1-8-11) grid, round-to-nearest-even."""
    u = np.ascontiguousarray(a, np.float32).view(np.uint32)
    lsb = (u >> np.uint32(12)) & np.uint32(1)
    r = (u + np.uint32(0x7FF) + lsb) & np.uint32(0xFFFFF000)
    return r.view(np.float32)


def _np_cast(a, kind):
    if kind == "bf16":
        return np.ascontiguousarray(np.asarray(a).astype(ml_dtypes.bfloat16))
    if kind == "f32r":
        return np.ascontiguousarray(_round_f32r(np.asarray(a)))
    return np.ascontiguousarray(np.asarray(a, np.float32))


def _routing_selector() -> tuple[np.ndarray, np.ndarray]:
    """AL/AR[l, n] in {0,1}: -log mixture[l] =
    sum_n AL[l, n] * softplus(z_n) + AR[l, n] * softplus(-z_n)."""
    AL = np.zeros((L, 16), np.float32)
    AR = np.zeros((L, 16), np.float32)
    for l in range(L):
        for d in range(DEPTH):
            j = l >> (DEPTH - d)          # node offset within level d
            n = (1 << d) - 1 + j          # global node index
            s = (l >> (DEPTH - 1 - d)) & 1  # 1 -> right (sigmoid branch)
            (AR if s else AL)[l, n] = 1.0
    return AL, AR


def build(cfg: Cfg):
    import concourse.mybir as mybir
    import concourse.tile as tile
    from concourse import bacc

    f32 = mybir.dt.float32
    AF = mybir.ActivationFunctionType
    MUL = mybir.AluOpType.mult

    def dt_of(kind):
        if kind == "bf16":
            return mybir.dt.bfloat16
        if kind == "f32r":
            return mybir.dt.float32r
        return f32

    x_dt = dt_of(cfg.mm1)
    a_dt = dt_of(cfg.mm2)

    # single token pass when aT is 2 bytes, else two passes (SBUF pressure)
    n_pass = 1 if cfg.mm2 == "bf16" else 2
    # DVE can read only ONE operand from PSUM: with the selector-matmul
    # broadcast (mp in PSUM) the relu output must land in SBUF first.
    relu_inplace = cfg.relu_inplace and cfg.bcast != "mm2"
    TPB = BC // n_pass
    TG = TPB // MOV
    TCC = TPB // P
    w2_full = cfg.mm2 == "bf16"   # whole [P, HC, O] leaf tiles resident

    nc = bacc.Bacc("TRN2", target_bir_lowering=False, debug=False,
                   num_devices=NCORES)

    x_d = nc.dram_tensor("x", [P, FC, BC], x_dt, kind="ExternalInput")
    nwt_d = nc.dram_tensor("nwt", [P, FC, 16], x_dt, kind="ExternalInput")
    nbp_d = nc.dram_tensor("nbp", [16, 1], f32, kind="ExternalInput")
    nbn_d = nc.dram_tensor("nbn", [16, 1], f32, kind="ExternalInput")
    atl_d = nc.dram_tensor("atl", [16, 16], f32, kind="ExternalInput")
    atr_d = nc.dram_tensor("atr", [16, 16], f32, kind="ExternalInput")
    w1_d = nc.dram_tensor("w1", [L, P, FC, H], x_dt, kind="ExternalInput")
    b1t_d = nc.dram_tensor("b1t", [P, L, HC], f32, kind="ExternalInput")
    w2_d = nc.dram_tensor("w2", [L, P, HC, O], a_dt, kind="ExternalInput")
    b2_d = nc.dram_tensor("b2", [L, O], a_dt, kind="ExternalInput")
    if cfg.bcast == "mm2":
        selm_d = nc.dram_tensor("selm", [16, L, P], a_dt, kind="ExternalInput")
    y_dt = mybir.dt.bfloat16 if cfg.y_bf16 else f32
    y_d = nc.dram_tensor("y", [BC, O], y_dt, kind="ExternalOutput")

    with tile.TileContext(nc) as tc:
        psum_h = 3 if cfg.bcast != "mm2" else 2
        with (
            tc.tile_pool(name="const", bufs=1) as const,
            tc.tile_pool(name="big", bufs=1) as big,
            tc.tile_pool(name="w1p", bufs=cfg.w1_bufs) as w1p,
            tc.tile_pool(name="w2p", bufs=cfg.w2_bufs) as w2p,
            tc.tile_pool(name="small", bufs=1) as small,
            tc.tile_pool(name="work", bufs=cfg.work_bufs) as work,
            tc.tile_pool(name="mbcp", bufs=cfg.mbc_bufs) as mbcp,
            tc.tile_pool(name="ph", bufs=psum_h, space="PSUM") as ph,
            tc.tile_pool(name="pr", bufs=1, space="PSUM") as pr,
            tc.tile_pool(name="py", bufs=3, space="PSUM") as py,
        ):
            # ---------------- constants ----------------
            nwt = const.tile([P, FC, 16], x_dt)
            nc.sync.dma_start(nwt, nwt_d[:])
            nbp = const.tile([16, 1], f32)
            nc.sync.dma_start(nbp, nbp_d[:])
            nbn = const.tile([16, 1], f32)
            nc.sync.dma_start(nbn, nbn_d[:])
            atl = const.tile([16, 16], f32)
            nc.sync.dma_start(atl, atl_d[:])
            atr = const.tile([16, 16], f32)
            nc.sync.dma_start(atr, atr_d[:])
            b1t = const.tile([P, L, HC], f32)
            nc.sync.dma_start(b1t, b1t_d[:])
            b2t = const.tile([L, O], a_dt)
            nc.sync.dma_start(b2t, b2_d[:])
            if cfg.bcast == "mm2":
                sel_m = const.tile([16, L, P], a_dt)
                nc.sync.dma_start(sel_m, selm_d[:])

            if cfg.diag_nodma:
                # timing diagnostic: stream nothing per-iteration; compute
                # against tiles loaded once (values wrong across leaves).
                xT_c = const.tile([P, FC, TPB], x_dt)
                nc.sync.dma_start(xT_c, x_d[:, :, 0:TPB])
                w1_c = const.tile([P, FC, H], x_dt)
                nc.sync.dma_start(w1_c, w1_d[0])
                w2_c = []
                for l in range(L):
                    w = w2p.tile([P, HC, O], a_dt, tag="w2")
                    nc.sync.dma_start(w, w2_d[l])
                    w2_c.append(w)

            def rep_body(_i=None):
                for ps in range(n_pass):
                    t0 = ps * TPB
                    if cfg.diag_nodma:
                        xT = xT_c
                    else:
                        xT = big.tile([P, FC, TPB], x_dt, tag="xT")
                        nc.sync.dma_start(xT, x_d[:, :, t0:t0 + TPB])
                    aT = big.tile([P, L, HC, TPB], a_dt, tag="aT")
                    mT = small.tile([16, TPB], f32, tag="mT")
                    mT_r = small.tile([16, TPB], a_dt, tag="mTr")

                    # ---- routing: mT[l, t] = prod of sigmoid gates ----
                    for g in range(TG):
                        gs = slice(g * MOV, (g + 1) * MOV)
                        lgT = pr.tile([16, MOV], f32, tag="lgT")
                        for fc in range(FC):
                            nc.tensor.matmul(
                                lgT, nwt[:, fc, :], xT[:, fc, gs],
                                start=(fc == 0), stop=(fc == FC - 1))
                        # softplus(+/-z) = ln(1 + exp(+/-z)); Softplus has no
                        # ACT table on this build, but exp+ln+relu share one.
                        sp_l = work.tile([16, MOV], f32, tag="spl")
                        nc.scalar.activation(sp_l, lgT, AF.Exp,
                                             bias=nbp, scale=1.0)
                        nc.scalar.activation(sp_l, sp_l, AF.Ln, bias=1.0)
                        sp_r = work.tile([16, MOV], f32, tag="spr")
                        nc.scalar.activation(sp_r, lgT, AF.Exp,
                                             bias=nbn, scale=-1.0)
                        nc.scalar.activation(sp_r, sp_r, AF.Ln, bias=1.0)
                        pmx = pr.tile([16, MOV], f32, tag="pmx")
                        nc.tensor.matmul(pmx, atl, sp_l, start=True, stop=False)
                        nc.tensor.matmul(pmx, atr, sp_r, start=False, stop=True)
                        nc.scalar.activation(mT[:, gs], pmx, AF.Exp, scale=-1.0)
                        nc.scalar.activation(mT_r[:, gs], pmx, AF.Exp,
                                             scale=-1.0)

                    # ---- phase 1: aT[l] = relu(w1.T x.T + b1) * mixture ----
                    for l in range(L):
                        if cfg.diag_nodma:
                            w1sb = w1_c
                        else:
                            w1sb = w1p.tile([P, FC, H], x_dt, tag="w1")
                            nc.sync.dma_start(w1sb, w1_d[l])
                        for g in range(TG):
                            gs = slice(g * MOV, (g + 1) * MOV)
                            if cfg.bcast == "gpsd":
                                mp = mbcp.tile([P, MOV], f32, tag="mbc")
                                nc.gpsimd.partition_broadcast(
                                    mp, mT[l:l + 1, gs])
                            elif cfg.bcast == "gps":
                                mrow = mbcp.tile([1, MOV], f32, tag="mrow")
                                nc.sync.dma_start(mrow, mT[l:l + 1, gs])
                                mp = mbcp.tile([P, MOV], f32, tag="mbc")
                                nc.gpsimd.partition_broadcast(mp, mrow)
                            else:
                                mp = pr.tile([P, MOV], f32, tag="pm")
                                nc.tensor.matmul(
                                    mp, sel_m[:, l, :], mT_r[:, gs],
                                    start=True, stop=True)
                            for hc in range(HC):
                                hp = ph.tile([P, MOV], f32, tag="ph")
                                for fc in range(FC):
                                    nc.tensor.matmul(
                                        hp,
                                        w1sb[:, fc, hc * P:(hc + 1) * P],
                                        xT[:, fc, gs],
                                        start=(fc == 0), stop=(fc == FC - 1))
                                if relu_inplace:
                                    nc.scalar.activation(
                                        hp, hp, AF.Relu,
                                        bias=b1t[:, l, hc:hc + 1])
                                    ar = hp
                                else:
                                    ar = work.tile([P, MOV], f32, tag="ar")
                                    nc.scalar.activation(
                                        ar, hp, AF.Relu,
                                        bias=b1t[:, l, hc:hc + 1])
                                nc.vector.tensor_tensor(
                                    aT[:, l, hc, gs], ar, mp, MUL)

                    # ---- phase 2: y = sum_l aT_l.T @ w2_l  (+ mixture @ b2) ----
                    if cfg.diag_nodma:
                        w2t = w2_c
                    elif w2_full:
                        w2t = []
                        for l in range(L):
                            w = w2p.tile([P, HC, O], a_dt, tag="w2")
                            nc.sync.dma_start(w, w2_d[l])
                            w2t.append(w)
                    for og in range(OG):
                        ogs = slice(og * MOV, (og + 1) * MOV)
                        if not w2_full:
                            w2t = []
                            for l in range(L):
                                w = w2p.tile([P, HC, MOV], a_dt, tag="w2")
                                nc.sync.dma_start(w, w2_d[l][:, :, ogs])
                                w2t.append(w)
                        for tcc in range(TCC):
                            tsl = slice(tcc * P, (tcc + 1) * P)
                            trow = slice(t0 + tcc * P, t0 + (tcc + 1) * P)
                            yp = py.tile([P, MOV], f32, tag="py")
                            nc.tensor.matmul(
                                yp, mT_r[:, tsl], b2t[:, ogs],
                                start=True, stop=False)
                            for l in range(L):
                                for hc in range(HC):
                                    wsl = (w2t[l][:, hc, ogs] if w2_full
                                           else w2t[l][:, hc, :])
                                    nc.tensor.matmul(
                                        yp, aT[:, l, hc, tsl], wsl,
                                        start=False,
                                        stop=(l == L - 1 and hc == HC - 1))
                            ye = work.tile([P, MOV], y_dt, tag="ye")
                            nc.vector.tensor_copy(ye, yp)
                            nc.sync.dma_start(y_d[trow, ogs], ye)

            if cfg.use_for_i:
                ET = mybir.EngineType
                hints = (ET.PE, ET.DVE, ET.Activation, ET.SP, ET.Pool)
                with tc.For_i(0, cfg.repeat, 1, hint_engines=hints) as _i:
                    rep_body(_i)
            else:
                for _ in range(cfg.repeat):
                    rep_body()

    nc.compile()
    return nc


def host_prep(inputs, cfg: Cfg):
    """Shared (replicated) parameter arrays in kernel layout."""
    node_weights = np.asarray(inputs["node_weights"], np.float32)
    node_biases = np.asarray(inputs["node_biases"], np.float32)
    w1s = np.asarray(inputs["w1s"], np.float32)
    b1s = np.asarray(inputs["b1s"], np.float32)
    w2s = np.asarray(inputs["w2s"], np.float32)
    b2s = np.asarray(inputs["b2s"], np.float32)

    nwt = np.zeros((P, FC, 16), np.float32)
    nwt[:, :, :NN] = node_weights.T.reshape(FC, P, NN).transpose(1, 0, 2)
    nb = np.zeros((16, 1), np.float32)
    nb[:NN, 0] = node_biases.reshape(NN)
    b1t = b1s.reshape(L, HC, P).transpose(2, 0, 1)              # [P, L, HC]
    # w1: [L, F, H] -> [L, P, FC, H] so each leaf DMA is contiguous/partition
    w1r = w1s.reshape(L, FC, P, H).transpose(0, 2, 1, 3)
    # w2: [L, H, O] -> [L, P, HC, O]
    w2r = w2s.reshape(L, HC, P, O).transpose(0, 2, 1, 3)
    out = {
        "nwt": _np_cast(nwt, cfg.mm1),
        "nbp": np.ascontiguousarray(nb),
        "nbn": np.ascontiguousarray(-nb),
        "atl": np.ascontiguousarray(_routing_selector()[0].T),
        "atr": np.ascontiguousarray(_routing_selector()[1].T),
        "w1": _np_cast(w1r, cfg.mm1),
        "b1t": np.ascontiguousarray(b1t),
        "w2": _np_cast(w2r, cfg.mm2),
        "b2": _np_cast(b2s, cfg.mm2),
    }
    if cfg.bcast == "mm2":
        sel = np.zeros((16, L, P), np.float32)
        sel[np.arange(L), np.arange(L), :] = 1.0
        out["selm"] = _np_cast(sel, cfg.mm2)
    return out


def prep_x(x, cfg: Cfg):
    """Per-core x shards in kernel layout [P, FC, BC]."""
    xc = _np_cast(np.asarray(x, np.float32), cfg.mm1)

    def shard(c):
        xs = xc[c * BC:(c + 1) * BC]
        return np.ascontiguousarray(xs.reshape(BC, FC, P).transpose(2, 1, 0))

    return [shard(c) for c in range(NCORES)]


def get_nc(cfg: Cfg):
    if cfg not in _BUILD_CACHE:
        _BUILD_CACHE[cfg] = build(cfg)
    return _BUILD_CACHE[cfg]


def run(inputs, cfg: Cfg):
    from concourse.bass_utils import run_bass_kernel_spmd

    nc = get_nc(cfg)
    params = host_prep(inputs, cfg)
    xshards = prep_x(inputs["x"], cfg)
    in_maps = [{"x": xshards[c], **params} for c in range(NCORES)]
    res = run_bass_kernel_spmd(nc, in_maps, core_ids=list(range(NCORES)))
    y = np.concatenate([np.asarray(res.results[c]["y"]).astype(np.float32)
                        for c in range(NCORES)], axis=0)
    return np.ascontiguousarray(y)


def kernel(**inputs) -> np.ndarray:
    return run(inputs, DEFAULT_CFG)
